# revision 13
# baseline (speedup 1.0000x reference)
"""LConv (7x7 position-linear conv) Trainium2 Bass kernel.

Full inputs in, full output out. Sharding: data-parallel over batch,
16 images -> 8 NeuronCores (2 images/core). abc/bias replicated.

Math: the 7x7 kernel weight is linear in position:
  w[u,v,c,o] = u*A[c,o] + v*B[c,o] + C[c,o]   (u,v in -3..3)
so with R = box7 along W of x and Q = box7 along H of x:
  out[o,i,j] = sum_u (u*A + C)[.,o] . R[., i+u, j]
             + sum_{v!=0} (v*B)[.,o] . Q[., i, j+v]  + bias[o]
13 matmul taps per 4-row output tile over two box-filtered maps; all
rhs views are row-contiguous (PE streams at ~N cycles per matmul).

R: sliding-box via the BOXDIFF custom-DVE op (cumsum(in0)-cumsum(in1))
   on the row-major stream; 7 lead zero cols per row make any
   row-aligned chunk self-contained.
Q: same op on a column-major (transposed) view, writing into a
   row-major Q buffer; 7 lead + 7 trail zero rows make every column
   page self-telescoping.

The image pair is processed as 4 half-image units (56 out rows each,
+/-3-row halo) so DMA, scans, and matmuls pipeline across units;
unit slot parity == top/bottom parity, so the static zero borders per
slot are set up once.
"""

import numpy as np

import concourse.bacc as bacc
import concourse.mybir as mybir
from concourse import tile
from concourse.bass_utils import run_bass_kernel_spmd

F32 = mybir.dt.float32
BF16 = mybir.dt.bfloat16
AF = mybir.ActivationFunctionType
ALU = mybir.AluOpType

B_TOT = 16
N_CORES = 8
B_PER = B_TOT // N_CORES
CIN = 128
COUT = 128
H = W = 112
PW2 = 122                  # 7 lead + 112 + 3 trail cols
UROWS = 56                 # output rows per unit (half image)
XROWS = 76                 # 7 lead + 62 (56+halo) + 7 trail rows
XBF = XROWS * PW2          # 9272
DROWS = 59                 # valid x rows DMA'd per unit
QK = XROWS - 7             # 69 scanned values per column page
OUT_ROWS = 4
OTF = OUT_ROWS * W         # 448
TPG = 7                    # psum tiles per group
GPU = UROWS // (OUT_ROWS * TPG)  # 2 groups per unit
QTAP_V = (-3, -2, -1, 1, 2, 3)
NTAPS = 7 + len(QTAP_V)    # 13
# R-scan chunks (row-aligned; chunk 0 covers all rows group 1 needs)
RCHUNKS = ((7, 34), (41, 28))

_CACHE = {}


def _register_opa():
    from concourse.dve_spec import Spec, Src0, Src1, scan, AluOp, lower
    import concourse.dve_ops as dve_ops
    from concourse.dve_uop import DveOpSpec

    if any(op.name == "BOXDIFF7" for op in dve_ops.OPS):
        return next(op for op in dve_ops.OPS if op.name == "BOXDIFF7")
    spec = Spec(
        body=scan(AluOp.ADD, Src0) - scan(AluOp.ADD, Src1),
        reference=lambda in0, in1: (
            np.cumsum(in0, axis=-1) - np.cumsum(in1, axis=-1)
        ),
    )
    row = dve_ops._CUSTOM_DVE_ROW_BASE + len(dve_ops.OPS)
    shas = {}
    for ver in ("v3", "v4"):
        s = DveOpSpec(
            name="BOXDIFF7", opcode=row, uops=lower(spec, ver=ver), rd1_en=True
        )
        shas[ver] = s.sha(ver)
    op = dve_ops.DveOp("BOXDIFF7", spec, subdim=False, uops_sha=shas)
    dve_ops.OPS.append(op)
    dve_ops._SUB_OPCODE_FOR_NAME[op.name] = row
    dve_ops.CUSTOM_DVE_SPECS[op.name] = op.spec
    return op


def _build():
    nc = bacc.Bacc("TRN2", target_bir_lowering=False, debug=False)
    opa = _register_opa()

    t_x = nc.dram_tensor("xs", [B_PER, CIN, H, W], F32, kind="ExternalInput")
    t_w = nc.dram_tensor("wts", [NTAPS, CIN, COUT], F32, kind="ExternalInput")
    t_bias = nc.dram_tensor("bias", [COUT, 1], F32, kind="ExternalInput")
    t_out = nc.dram_tensor("out", [B_PER, COUT, H, W], BF16, kind="ExternalOutput")

    with tile.TileContext(nc) as tc:
        with (
            tc.tile_pool(name="const", bufs=1) as cpool,
            tc.tile_pool(name="bufs", bufs=1) as bpool,
            tc.tile_pool(name="outs", bufs=4) as opool,
            tc.tile_pool(name="ps", bufs=1, space="PSUM") as ppool,
        ):
            # ---- constants ----
            wf = cpool.tile([CIN, NTAPS * COUT], F32, tag="wf")
            nc.scalar.dma_start(
                wf[:].rearrange("c (t o) -> c t o", t=NTAPS),
                t_w[:].transpose([1, 0, 2]),
            )
            wt = cpool.tile([CIN, NTAPS * COUT], BF16, tag="wt")
            nc.vector.tensor_copy(wt[:], wf[:])
            bias_sb = cpool.tile([COUT, 1], F32, tag="bias")
            nc.scalar.dma_start(bias_sb[:], t_bias[:])

            # ---- per-slot buffers (slot = unit parity = top/bottom) ----
            xbufs, rbufs, qbufs = [], [], []
            for s in range(2):
                xb = bpool.tile([CIN, XBF], F32, tag=f"xb{s}", name=f"xb{s}")
                xv = xb[:].rearrange("c (r q) -> c r q", q=PW2)
                nc.gpsimd.memset(xb[:, : 7 * PW2], 0.0)          # lead rows
                nc.gpsimd.memset(xb[:, (XROWS - 7) * PW2 :], 0.0)  # trail rows
                nc.gpsimd.memset(xv[:, 7 : XROWS - 7, 0:7], 0.0)   # lead cols
                nc.gpsimd.memset(xv[:, 7 : XROWS - 7, 7 + W :], 0.0)  # trail cols
                if s == 0:
                    nc.gpsimd.memset(xv[:, 7:10, :], 0.0)   # above-image pad
                else:
                    nc.gpsimd.memset(xv[:, 66:69, :], 0.0)  # below-image pad
                xbufs.append(xb)
                r = bpool.tile([CIN, XBF], BF16, tag=f"R{s}", name=f"R{s}")
                rbufs.append(r)
                # Qp: scan output, page(col)-major contiguous.
                # Qg: row-major relayout [56 k-rows x 118 cols], cols = gc 4..121.
                qp = bpool.tile([CIN, 115 * QK], BF16, tag=f"Qp{s}", name=f"Qp{s}")
                qg = bpool.tile([CIN, UROWS * 118], BF16, tag=f"Qg{s}", name=f"Qg{s}")
                qgv = qg[:].rearrange("c (k g) -> c k g", g=118)
                nc.gpsimd.memset(qgv[:, :, 115:118], 0.0)  # gc 119..121 zeros
                qbufs.append((qp, qg))

            def issue_in_dma(unit):
                # load a unit's x rows (4 slices; first two cover R chunk 0)
                b, hh = unit // 2, unit % 2
                xv = xbufs[hh][:].rearrange("c (r q) -> c r q", q=PW2)
                xr0 = max(0, 56 * hh - 3)            # first valid x row
                dst0 = 10 if hh == 0 else 7          # its XB row
                n0 = 41 - dst0                       # split at XB row 41
                n1 = DROWS - n0
                slices = ((0, n0 // 2), (n0 // 2, n0 - n0 // 2),
                          (n0, n1 // 2), (n0 + n1 // 2, n1 - n1 // 2))
                for si, (s0, ln) in enumerate(slices):
                    eng = nc.sync if si % 2 == 0 else nc.scalar
                    eng.dma_start(
                        xv[:, dst0 + s0 : dst0 + s0 + ln, 7 : 7 + W],
                        t_x[b, :, xr0 + s0 : xr0 + s0 + ln, :],
                    )

            _bank = [0]
            issue_in_dma(0)
            for unit in range(B_PER * 2):
                b, hh = unit // 2, unit % 2
                xb, R = xbufs[hh], rbufs[hh]
                qp, qg = qbufs[hh]
                xv = xb[:].rearrange("c (r q) -> c r q", q=PW2)
                xt = xb[:].rearrange("c (r q) -> c q r", q=PW2)  # [c,122,76]
                rv = R[:].rearrange("c (r q) -> c r q", q=PW2)
                qgv = qg[:].rearrange("c (k g) -> c k g", g=118)

                # ---- R chunk 0, Q scan, R chunk 1 ----
                def r_chunk(r0, nrows):
                    base = r0 * PW2
                    ln = nrows * PW2 - 7
                    nc.vector._custom_dve(
                        opa,
                        out=R[:, base : base + ln],
                        in0=xb[:, base + 7 : base + 7 + ln],
                        in1=xb[:, base : base + ln],
                    )

                r_chunk(*RCHUNKS[0])
                qpv = qp[:].rearrange("c (p k) -> c p k", k=QK)
                qpk = qp[:].rearrange("c (p k) -> c k p", k=QK)
                # Q scan in two page(column)-halves; relayout each half on
                # the scalar engine as soon as it lands (col-split halves)
                nc.vector._custom_dve(
                    opa, out=qpv[:, 0:58, :],
                    in0=xt[:, 4:62, 7:XROWS], in1=xt[:, 4:62, 0:QK],
                )
                nc.scalar.copy(qgv[:, :, 0:58], qpk[:, 6:62, 0:58])
                nc.vector._custom_dve(
                    opa, out=qpv[:, 58:115, :],
                    in0=xt[:, 62:119, 7:XROWS], in1=xt[:, 62:119, 0:QK],
                )
                r_chunk(*RCHUNKS[1])
                nc.scalar.copy(qgv[:, :, 58:115], qpk[:, 6:62, 58:115])
                # hoist next unit's input DMAs ahead of this unit's output
                # DMAs in the in-order Sync queue (outs wait on activations)
                if unit + 1 < B_PER * 2:
                    issue_in_dma(unit + 1)

                # ---- 13-tap matmuls; 4-tile groups rotate through all
                # 8 PSUM banks so consecutive groups use disjoint banks and
                # matmuls never wait on the previous group's activations ----
                for t0, ntiles in ((0, 4), (4, 4), (8, 4), (12, 2)):
                    accs = []
                    for t in range(ntiles):
                        bk = _bank[0]
                        _bank[0] = (bk + 1) % 8
                        accs.append(
                            ppool.tile(
                                [COUT, OTF], F32, tag=f"acc{bk}", name=f"acc{bk}"
                            )
                        )
                    for tap in range(NTAPS):
                        wslice = wt[:, tap * COUT : (tap + 1) * COUT]
                        for t in range(ntiles):
                            i0 = (t0 + t) * OUT_ROWS  # unit-local out row
                            if tap < 7:
                                u = tap - 3
                                rhs = rv[:, 10 + i0 + u : 14 + i0 + u, 3 : 3 + W]
                            else:
                                v = QTAP_V[tap - 7]
                                rhs = qgv[:, i0 : i0 + 4, 3 + v : 3 + v + 112]
                            nc.tensor.matmul(
                                accs[t][:],
                                wslice,
                                rhs,
                                start=(tap == 0),
                                stop=(tap == NTAPS - 1),
                            )
                    for t in range(ntiles):
                        i0 = (t0 + t) * OUT_ROWS
                        ot = opool.tile([COUT, OTF], BF16, tag="ot", name="ot")
                        if t % 2 == 0:
                            nc.scalar.activation(
                                ot[:], accs[t][:], AF.Identity,
                                bias=bias_sb[:], scale=1.0,
                            )
                        else:
                            nc.vector.tensor_scalar_add(
                                ot[:], accs[t][:], bias_sb[:]
                            )
                        nc.sync.dma_start(
                            t_out[
                                b, :, 56 * hh + i0 : 56 * hh + i0 + OUT_ROWS, :
                            ].rearrange("o r j -> o (r j)"),
                            ot[:],
                        )

    nc.compile()
    return nc


def _make_in_maps(x, abc, bias):
    A, Bm, Cc = abc[0:128], abc[128:256], abc[256:384]
    taps = [u * A + Cc for u in range(-3, 4)] + [v * Bm for v in QTAP_V]
    wts = np.ascontiguousarray(np.stack(taps), dtype=np.float32)
    bias2 = np.ascontiguousarray(bias.reshape(COUT, 1), dtype=np.float32)
    return [
        {
            "xs": np.ascontiguousarray(x[c * B_PER : (c + 1) * B_PER]),
            "wts": wts,
            "bias": bias2,
        }
        for c in range(N_CORES)
    ]


def kernel(x: np.ndarray, abc: np.ndarray, bias: np.ndarray) -> np.ndarray:
    x = np.ascontiguousarray(x, dtype=np.float32)
    abc = np.asarray(abc, dtype=np.float32)
    bias = np.asarray(bias, dtype=np.float32)

    if "nc" not in _CACHE:
        _CACHE["nc"] = _build()
    nc = _CACHE["nc"]

    in_maps = _make_in_maps(x, abc, bias)
    res = run_bass_kernel_spmd(nc, in_maps, list(range(N_CORES)))
    out = np.concatenate(
        [np.asarray(res.results[c]["out"]) for c in range(N_CORES)], axis=0
    )
    return out.astype(np.float32)


if __name__ == "__main__":
    rng = np.random.default_rng(0)
    x = rng.standard_normal((16, 128, 112, 112), dtype=np.float32)
    abc = (rng.standard_normal((384, 128)) * 0.05).astype(np.float32)
    bias = (rng.standard_normal((128,)) * 0.05).astype(np.float32)
    out = kernel(x=x, abc=abc, bias=bias)
    print(out.shape, out.dtype)


# revision 14
# speedup vs baseline: 1.0815x; 1.0815x over previous
"""LConv (7x7 position-linear conv) Trainium2 Bass kernel.

Full inputs in, full output out. Sharding: data-parallel over batch,
16 images -> 8 NeuronCores (2 images/core). abc/bias replicated.

Math: the 7x7 kernel weight is linear in position:
  w[u,v,c,o] = u*A[c,o] + v*B[c,o] + C[c,o]   (u,v in -3..3)
so with R = box7 along W of x and Q = box7 along H of x:
  out[o,i,j] = sum_u (u*A + C)[.,o] . R[., i+u, j]
             + sum_{v!=0} (v*B)[.,o] . Q[., i, j+v]  + bias[o]
13 matmul taps per 4-row output tile over two box-filtered maps; all
rhs views are row-contiguous (PE streams at ~N cycles per matmul).

R: sliding-box via the BOXDIFF custom-DVE op (cumsum(in0)-cumsum(in1))
   on the row-major stream; 7 lead zero cols per row make any
   row-aligned chunk self-contained.
Q: same op on a column-major (transposed) view, writing into a
   row-major Q buffer; 7 lead + 7 trail zero rows make every column
   page self-telescoping.

The image pair is processed as 4 half-image units (56 out rows each,
+/-3-row halo) so DMA, scans, and matmuls pipeline across units;
unit slot parity == top/bottom parity, so the static zero borders per
slot are set up once.
"""

import numpy as np

import concourse.bacc as bacc
import concourse.mybir as mybir
from concourse import tile
from concourse.bass_utils import run_bass_kernel_spmd

F32 = mybir.dt.float32
BF16 = mybir.dt.bfloat16
AF = mybir.ActivationFunctionType
ALU = mybir.AluOpType

B_TOT = 16
N_CORES = 8
B_PER = B_TOT // N_CORES
CIN = 128
COUT = 128
H = W = 112
PW2 = 122                  # 7 lead + 112 + 3 trail cols
UROWS = 56                 # output rows per unit (half image)
XROWS = 76                 # 7 lead + 62 (56+halo) + 7 trail rows
XBF = XROWS * PW2          # 9272
DROWS = 59                 # valid x rows DMA'd per unit
QK = XROWS - 7             # 69 scanned values per column page
OUT_ROWS = 4
OTF = OUT_ROWS * W         # 448
TPG = 7                    # psum tiles per group
GPU = UROWS // (OUT_ROWS * TPG)  # 2 groups per unit
QTAP_V = (-3, -2, -1, 1, 2, 3)
NTAPS = 7 + len(QTAP_V)    # 13
# R-scan chunks (row-aligned; chunk 0 covers all rows group 1 needs)
RCHUNKS = ((7, 34), (41, 28))

_CACHE = {}


def _register_opa():
    from concourse.dve_spec import Spec, Src0, Src1, scan, AluOp, lower
    import concourse.dve_ops as dve_ops
    from concourse.dve_uop import DveOpSpec

    if any(op.name == "BOXDIFF7" for op in dve_ops.OPS):
        return next(op for op in dve_ops.OPS if op.name == "BOXDIFF7")
    spec = Spec(
        body=scan(AluOp.ADD, Src0) - scan(AluOp.ADD, Src1),
        reference=lambda in0, in1: (
            np.cumsum(in0, axis=-1) - np.cumsum(in1, axis=-1)
        ),
    )
    row = dve_ops._CUSTOM_DVE_ROW_BASE + len(dve_ops.OPS)
    shas = {}
    for ver in ("v3", "v4"):
        s = DveOpSpec(
            name="BOXDIFF7", opcode=row, uops=lower(spec, ver=ver), rd1_en=True
        )
        shas[ver] = s.sha(ver)
    op = dve_ops.DveOp("BOXDIFF7", spec, subdim=False, uops_sha=shas)
    dve_ops.OPS.append(op)
    dve_ops._SUB_OPCODE_FOR_NAME[op.name] = row
    dve_ops.CUSTOM_DVE_SPECS[op.name] = op.spec
    return op


def _build():
    nc = bacc.Bacc("TRN2", target_bir_lowering=False, debug=False)
    opa = _register_opa()

    t_x = nc.dram_tensor("xs", [B_PER, CIN, H, W], F32, kind="ExternalInput")
    t_w = nc.dram_tensor("wts", [NTAPS, CIN, COUT], F32, kind="ExternalInput")
    t_bias = nc.dram_tensor("bias", [COUT, 1], F32, kind="ExternalInput")
    t_out = nc.dram_tensor("out", [B_PER, COUT, H, W], BF16, kind="ExternalOutput")

    with tile.TileContext(nc) as tc:
        with (
            tc.tile_pool(name="const", bufs=1) as cpool,
            tc.tile_pool(name="bufs", bufs=1) as bpool,
            tc.tile_pool(name="outs", bufs=4) as opool,
            tc.tile_pool(name="ps", bufs=1, space="PSUM") as ppool,
        ):
            # ---- constants ----
            wf = cpool.tile([CIN, NTAPS * COUT], F32, tag="wf")
            nc.scalar.dma_start(
                wf[:].rearrange("c (t o) -> c t o", t=NTAPS),
                t_w[:].transpose([1, 0, 2]),
            )
            wt = cpool.tile([CIN, NTAPS * COUT], BF16, tag="wt")
            nc.vector.tensor_copy(wt[:], wf[:])
            bias_sb = cpool.tile([COUT, 1], F32, tag="bias")
            nc.scalar.dma_start(bias_sb[:], t_bias[:])

            # ---- per-slot buffers (slot = unit parity = top/bottom) ----
            xbufs, rbufs, qbufs = [], [], []
            for s in range(2):
                xb = bpool.tile([CIN, XBF], F32, tag=f"xb{s}", name=f"xb{s}")
                xv = xb[:].rearrange("c (r q) -> c r q", q=PW2)
                nc.gpsimd.memset(xb[:, : 7 * PW2], 0.0)          # lead rows
                nc.gpsimd.memset(xb[:, (XROWS - 7) * PW2 :], 0.0)  # trail rows
                nc.gpsimd.memset(xv[:, 7 : XROWS - 7, 0:7], 0.0)   # lead cols
                nc.gpsimd.memset(xv[:, 7 : XROWS - 7, 7 + W :], 0.0)  # trail cols
                if s == 0:
                    nc.gpsimd.memset(xv[:, 7:10, :], 0.0)   # above-image pad
                else:
                    nc.gpsimd.memset(xv[:, 66:69, :], 0.0)  # below-image pad
                xbufs.append(xb)
                r = bpool.tile([CIN, XBF], BF16, tag=f"R{s}", name=f"R{s}")
                rbufs.append(r)
                # Qp: scan output, page(col)-major contiguous.
                # Qg: row-major relayout [56 k-rows x 118 cols], cols = gc 4..121.
                qp = bpool.tile([CIN, 115 * QK], BF16, tag=f"Qp{s}", name=f"Qp{s}")
                qg = bpool.tile([CIN, UROWS * 118], BF16, tag=f"Qg{s}", name=f"Qg{s}")
                qgv = qg[:].rearrange("c (k g) -> c k g", g=118)
                nc.gpsimd.memset(qgv[:, :, 115:118], 0.0)  # gc 119..121 zeros
                qbufs.append((qp, qg))

            def issue_in_dma(unit):
                # load a unit's x rows (4 slices; first two cover R chunk 0)
                b, hh = unit // 2, unit % 2
                xv = xbufs[hh][:].rearrange("c (r q) -> c r q", q=PW2)
                xr0 = max(0, 56 * hh - 3)            # first valid x row
                dst0 = 10 if hh == 0 else 7          # its XB row
                n0 = 41 - dst0                       # split at XB row 41
                n1 = DROWS - n0
                slices = ((0, n0 // 2), (n0 // 2, n0 - n0 // 2),
                          (n0, n1 // 2), (n0 + n1 // 2, n1 - n1 // 2))
                for si, (s0, ln) in enumerate(slices):
                    eng = nc.sync if si % 2 == 0 else nc.scalar
                    eng.dma_start(
                        xv[:, dst0 + s0 : dst0 + s0 + ln, 7 : 7 + W],
                        t_x[b, :, xr0 + s0 : xr0 + s0 + ln, :],
                    )

            _bank = [0]
            issue_in_dma(0)
            for unit in range(B_PER * 2):
                b, hh = unit // 2, unit % 2
                xb, R = xbufs[hh], rbufs[hh]
                qp, qg = qbufs[hh]
                xv = xb[:].rearrange("c (r q) -> c r q", q=PW2)
                xt = xb[:].rearrange("c (r q) -> c q r", q=PW2)  # [c,122,76]
                rv = R[:].rearrange("c (r q) -> c r q", q=PW2)
                qgv = qg[:].rearrange("c (k g) -> c k g", g=118)

                # ---- R chunk 0, Q scan, R chunk 1 ----
                def r_chunk(r0, nrows):
                    base = r0 * PW2
                    ln = nrows * PW2 - 7
                    nc.vector._custom_dve(
                        opa,
                        out=R[:, base : base + ln],
                        in0=xb[:, base + 7 : base + 7 + ln],
                        in1=xb[:, base : base + ln],
                    )

                r_chunk(*RCHUNKS[0])
                qpv = qp[:].rearrange("c (p k) -> c p k", k=QK)
                qpk = qp[:].rearrange("c (p k) -> c k p", k=QK)
                # Q scan in two page(column)-halves; relayout each half on
                # the scalar engine as soon as it lands (col-split halves)
                nc.vector._custom_dve(
                    opa, out=qpv[:, 0:58, :],
                    in0=xt[:, 4:62, 7:XROWS], in1=xt[:, 4:62, 0:QK],
                )
                nc.scalar.copy(qgv[:, :, 0:58], qpk[:, 6:62, 0:58])
                nc.vector._custom_dve(
                    opa, out=qpv[:, 58:115, :],
                    in0=xt[:, 62:119, 7:XROWS], in1=xt[:, 62:119, 0:QK],
                )
                r_chunk(*RCHUNKS[1])
                nc.scalar.copy(qgv[:, :, 58:115], qpk[:, 6:62, 58:115])
                # hoist next unit's input DMAs ahead of this unit's output
                # DMAs in the in-order Sync queue (outs wait on activations)
                if unit + 1 < B_PER * 2:
                    issue_in_dma(unit + 1)

                # ---- 13-tap matmuls; 4-tile groups rotate through all
                # 8 PSUM banks so consecutive groups use disjoint banks and
                # matmuls never wait on the previous group's activations ----
                for t0, ntiles in ((0, 4), (4, 4), (8, 4), (12, 2)):
                    accs = []
                    for t in range(ntiles):
                        bk = _bank[0]
                        _bank[0] = (bk + 1) % 8
                        accs.append(
                            ppool.tile(
                                [COUT, OTF], F32, tag=f"acc{bk}", name=f"acc{bk}"
                            )
                        )
                    for tap in range(NTAPS):
                        wslice = wt[:, tap * COUT : (tap + 1) * COUT]
                        for t in range(ntiles):
                            i0 = (t0 + t) * OUT_ROWS  # unit-local out row
                            if tap < 7:
                                u = tap - 3
                                rhs = rv[:, 10 + i0 + u : 14 + i0 + u, 3 : 3 + W]
                            else:
                                v = QTAP_V[tap - 7]
                                rhs = qgv[:, i0 : i0 + 4, 3 + v : 3 + v + 112]
                            nc.tensor.matmul(
                                accs[t][:],
                                wslice,
                                rhs,
                                start=(tap == 0),
                                stop=(tap == NTAPS - 1),
                            )
                    for t in range(ntiles):
                        i0 = (t0 + t) * OUT_ROWS
                        ot = opool.tile([COUT, OTF], BF16, tag="ot", name="ot")
                        nc.scalar.activation(
                            ot[:], accs[t][:], AF.Identity,
                            bias=bias_sb[:], scale=1.0,
                        )
                        nc.sync.dma_start(
                            t_out[
                                b, :, 56 * hh + i0 : 56 * hh + i0 + OUT_ROWS, :
                            ].rearrange("o r j -> o (r j)"),
                            ot[:],
                        )

    nc.compile()
    return nc


def _make_in_maps(x, abc, bias):
    A, Bm, Cc = abc[0:128], abc[128:256], abc[256:384]
    taps = [u * A + Cc for u in range(-3, 4)] + [v * Bm for v in QTAP_V]
    wts = np.ascontiguousarray(np.stack(taps), dtype=np.float32)
    bias2 = np.ascontiguousarray(bias.reshape(COUT, 1), dtype=np.float32)
    return [
        {
            "xs": np.ascontiguousarray(x[c * B_PER : (c + 1) * B_PER]),
            "wts": wts,
            "bias": bias2,
        }
        for c in range(N_CORES)
    ]


def kernel(x: np.ndarray, abc: np.ndarray, bias: np.ndarray) -> np.ndarray:
    x = np.ascontiguousarray(x, dtype=np.float32)
    abc = np.asarray(abc, dtype=np.float32)
    bias = np.asarray(bias, dtype=np.float32)

    if "nc" not in _CACHE:
        _CACHE["nc"] = _build()
    nc = _CACHE["nc"]

    in_maps = _make_in_maps(x, abc, bias)
    res = run_bass_kernel_spmd(nc, in_maps, list(range(N_CORES)))
    out = np.concatenate(
        [np.asarray(res.results[c]["out"]) for c in range(N_CORES)], axis=0
    )
    return out.astype(np.float32)


if __name__ == "__main__":
    rng = np.random.default_rng(0)
    x = rng.standard_normal((16, 128, 112, 112), dtype=np.float32)
    abc = (rng.standard_normal((384, 128)) * 0.05).astype(np.float32)
    bias = (rng.standard_normal((128,)) * 0.05).astype(np.float32)
    out = kernel(x=x, abc=abc, bias=bias)
    print(out.shape, out.dtype)
